# revision 2
# baseline (speedup 1.0000x reference)
"""Trainium2 Bass kernel for nn_CNNseq_15564961481149 (dense_cnn).

Computes: embed lookup -> 3 parallel 1-D convs (K=3,4,5, channels-first)
-> bias -> max-over-time -> concat -> relu, for text [16, 64, 128] over a
[30000, 512] embedding table, F=256 filters per conv.

Strategy (pure data parallel over 8 NeuronCores):
  - Flatten B*S = 1024 samples; 128 samples per core.
  - Embedding table converted to fp16 on host, gathered on-device with
    dma_gather(transpose=True): the gathered tile lands as
    [d%128 (partition), d//128 (chunk), token] -- exactly the moving-operand
    layout the PE needs (contraction dim on partitions).
  - Conv-as-matmul: for each tap j and d-chunk c, a [128d, 128f] stationary
    tile multiplies the token columns shifted by j; accumulated in PSUM over
    all (c, j).  4 samples per matmul via a 2-level free dim (4 x L_out <= 512).
  - max-over-time on DVE straight out of PSUM, bias+relu folded in after
    (max(y + b) == max(y) + b), PE-transpose of the [f, sample] result so the
    final DMA to DRAM is fully contiguous.

Inputs are bf16-quantized for the matmul (fp32 accumulate in PSUM); host-
simulated end-to-end max relative error 4.5e-3 (gate 2e-2).  bf16 replaced
fp16 in the final session: the PE streams bf16 matmuls 25% faster (drift-
immune interleaved A/B: 733 vs 974 us) -- do not "fix" the dtype back.

This structure measures at the architectural floor for the direct algorithm
(HW-measured via repeat-loop marginals, one optimization session spent
trying to beat it):
  - Per-core PE row time: 3072 matmuls x ~500 rows x 1 cyc/row @ 2.4 GHz
    = 638 us.  Instruction count is minimal: free dim (504) is capped by
    the 2 KB PSUM bank, contraction (128) by partitions, out-partitions
    (128) force the m-split.  Measured ~278 ns/matmul = 208 row-time
    + ~62 LDWEIGHTS + ~8 modeled seq overhead.
  - LDWEIGHTS is emitted per-matmul by legalization (LDW+MM pairs,
    ldweights=False on the MM already); cost is ~2 B/cycle/partition and
    CANNOT be elided: repeated-stationary streaks measured SLOWER (+95
    ns/inst), ldweights-field suppression is ignored, and stationary
    reuse across gather tiles (T=2/4 interleave) pays a larger PSUM
    bank-switch penalty (+22..45 ns/inst) than the LDW it saves.
  - fp8e4 DoubleRow (256-contraction/inst) runs at 1 cyc/row on HW (2x
    fp16 MACs/inst; the CoreSim 0.5 cyc/row model is optimistic), but
    plain fp8 fails the 2e-2 gate (~8e-2) and error-corrected fp8
    (wh@xh + wh@xl + wl@xh, all-scale-1024 single PSUM group, rel err
    2.9e-3) needs 1.5x the MACs -> measured 1121-1530 us.  Dead.
  - Interleaved token order (f = 4t+s; gives 3-dim DR moving APs) makes
    the max-reduce stride-4 over PSUM: +53 ns/matmul equivalent.  Dead.
  - The c-outer/j-inner matmul order below is load-bearing: consecutive
    instructions then read overlapping windows of the SAME x chunk.
    j-outer/c-inner (chunk hop every instruction) measured 6.1% slower
    in a drift-immune interleaved A/B.  Keep c outer.
  - Axon HW timing drifts ~60% across hours (same binary: 503-977 us);
    only back-to-back or interleaved comparisons are meaningful.
  - Winograd F(2,3)/F(4,3) would cut PE rows 1.5-2.3x but the input/
    output transforms land ~300-600 us on DVE/Act; marginal at best.

Second optimization session: vocab-precompute table (T = embed @ W_cat,
conv -> 6KB/token dma_gather + DVE tap-sum/max) investigated end-to-end
and DEAD on this platform; measured facts for the record (probe*.py,
kernel2/3.py, _transcript):
  - addr_space="Shared" DRAM is shared only within an HBM PAIR (cores
    2k,2k+1) on trn2 LNC1 (probe-verified by racing writes).  8-way table
    sharing needs ~140MB/pair cross-pair replication -> dead; pair-level
    dedup (32768 inst -> ~19.9K uniques) only cuts PE to ~410us/core and
    core-local dedup (16384 -> ~12.6K) to ~510us -- TimelineSim marginal
    ~710us, no better than this direct kernel.
  - dma_gather(transpose=True, 6KB rows): 183 GB/s on 1 SWDGE queue, 259
    GB/s round-robin on 4 (num_swdge_queues=4); transpose=False reaches
    343; plain DMA 389.  The 100MB/core table gather alone is ~390us.
  - Manual .then_inc/wait_ge semaphores inside TileContext hang the
    device (probe D3/D4); ordering must use add_dep_helper edges or AP
    deps.  Raw-bass (no TC) sems work (probe P1).
  - dma_start(cond=reg) on the SP HWDGE queue wedges the device (probe
    E1); needed for SPMD pair-half write offsets.  gpsimd/SWDGE cond
    untested-clean (E2 ran on the wedged device).  Without it, SPMD
    cores cannot write disjoint halves of a pair-Shared table.
  - Pair AllReduce [[0,1],[2,3],[4,5],[6,7]] works inside TC if NO dep
    edges/then_inc touch the collective instruction itself; order via
    AP deps on its in/out DRAM buffers (probe C).
"""

from contextlib import ExitStack

import numpy as np

import concourse.mybir as mybir
import concourse.tile as tile
from concourse import bacc
from concourse.bass_utils import run_bass_kernel_spmd
from concourse.masks import make_identity

# Problem constants (hardcoded per harness contract).
B, S, L, D, F, V = 16, 64, 128, 512, 256, 30000
N_CORES = 8
NSAMP = B * S // N_CORES          # 128 samples per core
KS = (3, 4, 5)                    # conv kernel sizes
SLOT_BASE = (0, 3, 7)             # tap-slot offsets for conv3/4/5 (12 total)
N_SLOTS = 12
SPG = 4                           # samples per gather tile (512 idxs; >512 crashes the SWDGE transpose-gather)
NGT = NSAMP // SPG                # gather tiles per core
GPT = SPG // 4                    # groups of 4 samples per gather tile

f16 = mybir.dt.bfloat16  # bf16: PE streams it 25% faster than fp16 (interleaved A/B 733 vs 974us); rel err 4.5e-3 vs gate 2e-2
f32 = mybir.dt.float32
i16 = mybir.dt.int16


def build_nc(nsamp=NSAMP, spg=SPG, mode="full"):
    """Build the per-core Bass program (SPMD: same program, 8 cores).

    mode: "full" | "nogather" (memset x tiles) | "nomm" (skip matmul/reduce)
    -- the reduced modes exist only to attribute wall-clock time.
    """
    ngt = nsamp // spg
    gpt = spg // 4
    t_tot = nsamp * L                  # tokens per core
    n_idx = spg * L                    # tokens per gather
    idx_cols_per_gather = n_idx // 16

    nc = bacc.Bacc("TRN2", target_bir_lowering=False, debug=False,
                   num_devices=N_CORES)

    emb_h = nc.dram_tensor("emb", [V, D], f16, kind="ExternalInput")
    idx_h = nc.dram_tensor("idx", [128, t_tot // 16], i16, kind="ExternalInput")
    wst_h = nc.dram_tensor("wst", [128, N_SLOTS, 4, 2, 128], f16,
                           kind="ExternalInput")
    bias_h = nc.dram_tensor("bias", [128, 6], f32, kind="ExternalInput")
    out_h = nc.dram_tensor("out", [nsamp, 3 * F], f32, kind="ExternalOutput")

    with tile.TileContext(nc) as tc, ExitStack() as ctx:
        cpool = ctx.enter_context(tc.tile_pool(name="consts", bufs=1))
        xpool = ctx.enter_context(tc.tile_pool(name="x", bufs=6))
        pspool = ctx.enter_context(
            tc.tile_pool(name="ps", bufs=6, space="PSUM"))
        tppool = ctx.enter_context(
            tc.tile_pool(name="tp", bufs=2, space="PSUM"))

        idx_sb = cpool.tile([128, t_tot // 16], i16)
        w_sb = cpool.tile([128, N_SLOTS, 4, 2, 128], f16)
        bias_sb = cpool.tile([128, 6], f32)
        ident = cpool.tile([128, 128], f32)
        out_sb = cpool.tile([128, 6, nsamp], f32)
        out_t = cpool.tile([nsamp, 6 * 128], f32)

        nc.sync.dma_start(out=idx_sb[:], in_=idx_h.ap()[:])
        nc.sync.dma_start(out=w_sb[:], in_=wst_h.ap()[:])
        nc.sync.dma_start(out=bias_sb[:], in_=bias_h.ap()[:])
        make_identity(nc, ident[:])

        if mode == "nomm":
            nc.gpsimd.memset(out_sb[:], 0.0)
        reps = int(mode[len("repeat"):]) if mode.startswith("repeat") else 0
        loop_cm = tc.For_i(0, reps, 1) if reps else None
        if loop_cm is not None:
            loop_cm.__enter__()
        # Process gather tiles in batches of QB so each stationary weight tile
        # is reused across QB matmuls (amortizes LDWEIGHTS 4x).
        # QB>1 (stationary reuse across gather tiles) measured ~28% SLOWER on
        # HW than back-to-back same-bank matmuls: LDWEIGHTS is already hidden
        # by the PE reorder window + dual SBUF read ports, and interleaving
        # PSUM banks/operand buffers per instruction costs more than it saves.
        QB = 1
        for sup in range(ngt // QB):
            xvs = []
            for q in range(QB):
                t = sup * QB + q
                xt = xpool.tile([128, 4, n_idx], f16, tag="xt")
                if mode == "nogather":
                    nc.gpsimd.memset(xt[:], 0.0)
                else:
                    nc.gpsimd.dma_gather(
                        out_ap=xt[:],
                        in_ap=emb_h.ap()[:],
                        idxs_ap=idx_sb[:, t * idx_cols_per_gather:
                                       (t + 1) * idx_cols_per_gather],
                        num_idxs=n_idx,
                        num_idxs_reg=n_idx,
                        elem_size=D,
                        transpose=True,
                    )
                if mode == "nomm":
                    nc.vector.tensor_copy(out_sb[:, 0, t:t + 1], xt[:, 0, :1])
                xvs.append(xt.rearrange("p c (s l) -> p c s l", s=spg))
            if mode == "nomm":
                continue
            for k_idx, K in enumerate(KS):
                lout = L - K + 1
                for m in range(2):
                    pss = [pspool.tile([128, 4, lout], f32, tag="ps",
                                       name=f"ps_{sup}_{k_idx}_{m}_{q}")
                           for q in range(QB)]
                    n_mm = 4 * K
                    mm = 0
                    for c in range(4):
                        for j in range(K):
                            for q in range(QB):
                                nc.tensor.matmul(
                                    pss[q][:],
                                    w_sb[:, SLOT_BASE[k_idx] + j, c, m, :],
                                    xvs[q][:, c, 0:4, j:j + lout],
                                    start=(mm == 0),
                                    stop=(mm == n_mm - 1),
                                )
                            mm += 1
                    tile6 = k_idx * 2 + m
                    for q in range(QB):
                        gidx = sup * QB + q
                        nc.vector.reduce_max(
                            out_sb[:, tile6, gidx * 4:gidx * 4 + 4],
                            pss[q][:],
                            axis=mybir.AxisListType.X,
                        )

        if loop_cm is not None:
            loop_cm.__exit__(None, None, None)
        # bias + relu on [f(partition), sample] layout, then PE-transpose so
        # the final DMA writes contiguous [sample, 768] rows.
        for tile6 in range(6):
            nc.vector.tensor_scalar(
                out_sb[:, tile6, :], out_sb[:, tile6, :],
                bias_sb[:, tile6:tile6 + 1], 0.0,
                op0=mybir.AluOpType.add, op1=mybir.AluOpType.max,
            )
            tp = tppool.tile([nsamp, 128], f32, tag="tp")
            nc.tensor.transpose(tp[:], out_sb[:, tile6, :], ident[:])
            nc.vector.tensor_copy(
                out_t[:, tile6 * 128:(tile6 + 1) * 128], tp[:])
        nc.sync.dma_start(out=out_h.ap()[:], in_=out_t[:])

    nc.compile()
    return nc


def prep_inputs(text, embed, w3, b3, w4, b4, w5, b5, nsamp=NSAMP, spg=SPG,
                n_cores=N_CORES):
    """Host-side marshaling: shard text, wrap gather indices, fp16-quantize
    and retile the weights/embedding."""
    text = np.ascontiguousarray(np.asarray(text).reshape(B * S, L))
    assert text.max() < V and text.min() >= 0
    import ml_dtypes
    emb16 = np.ascontiguousarray(np.asarray(embed, dtype=np.float32).astype(ml_dtypes.bfloat16))

    wst = np.zeros((128, N_SLOTS, 4, 2, 128), np.float32)
    for k_idx, w in enumerate((w3, w4, w5)):
        w = np.asarray(w, dtype=np.float32)
        for j in range(KS[k_idx]):
            # wst[dd, slot, c, m, ff] = w[m*128+ff, c*128+dd, j]
            wj = w[:, :, j].reshape(2, 128, 4, 128)      # [m, ff, c, dd]
            wst[:, SLOT_BASE[k_idx] + j] = wj.transpose(3, 2, 0, 1)
    wst = np.ascontiguousarray(wst.astype(ml_dtypes.bfloat16))

    bias = np.zeros((128, 6), np.float32)
    for k_idx, b in enumerate((b3, b4, b5)):
        bias[:, 2 * k_idx:2 * k_idx + 2] = \
            np.asarray(b, dtype=np.float32).reshape(2, 128).T
    bias = np.ascontiguousarray(bias)

    ngt = nsamp // spg
    in_maps = []
    for r in range(n_cores):
        tcore = text[r * nsamp:(r + 1) * nsamp].astype(np.int16)
        # token i of gather tile t -> partition i%16, column t*(spg*L/16)+i//16;
        # the 16-row block must be replicated to all 128 partitions (each of
        # the 8 gpsimd sub-cores reads its own 16-partition stripe).
        a = tcore.reshape(ngt, spg * L // 16, 16)         # [t, c, p]
        idx = np.tile(a.transpose(2, 0, 1).reshape(16, -1), (8, 1))
        in_maps.append({
            "emb": emb16,
            "idx": np.ascontiguousarray(idx),
            "wst": wst,
            "bias": bias,
        })
    return in_maps


_CACHE = {}


def kernel(text, embed, w3, b3, w4, b4, w5, b5):
    if "nc" not in _CACHE:
        _CACHE["nc"] = build_nc()
    nc = _CACHE["nc"]
    in_maps = prep_inputs(text, embed, w3, b3, w4, b4, w5, b5)
    res = run_bass_kernel_spmd(nc, in_maps, list(range(N_CORES)))
    out = np.concatenate([res.results[r]["out"] for r in range(N_CORES)],
                         axis=0)
    return out.reshape(B, S, 3 * F).astype(np.float32)



# revision 3
# speedup vs baseline: 1.1843x; 1.1843x over previous
"""Trainium2 Bass kernel for nn_CNNseq_15564961481149 (dense_cnn).

Computes: embed lookup -> 3 parallel 1-D convs (K=3,4,5, channels-first)
-> bias -> max-over-time -> concat -> relu, for text [16, 64, 128] over a
[30000, 512] embedding table, F=256 filters per conv.

Strategy (pure data parallel over 8 NeuronCores):
  - Flatten B*S = 1024 samples; 128 samples per core.
  - Embedding table converted to fp16 on host, gathered on-device with
    dma_gather(transpose=True): the gathered tile lands as
    [d%128 (partition), d//128 (chunk), token] -- exactly the moving-operand
    layout the PE needs (contraction dim on partitions).
  - Conv-as-matmul: for each tap j and d-chunk c, a [128d, 128f] stationary
    tile multiplies the token columns shifted by j; accumulated in PSUM over
    all (c, j).  4 samples per matmul via a 2-level free dim (4 x L_out <= 512).
  - max-over-time on DVE straight out of PSUM, bias+relu folded in after
    (max(y + b) == max(y) + b), PE-transpose of the [f, sample] result so the
    final DMA to DRAM is fully contiguous.

Inputs are bf16-quantized for the matmul (fp32 accumulate in PSUM); host-
simulated end-to-end max relative error 4.5e-3 (gate 2e-2).  bf16 replaced
fp16 in the final session: the PE streams bf16 matmuls 25% faster (drift-
immune interleaved A/B: 733 vs 974 us) -- do not "fix" the dtype back.

This structure measures at the architectural floor for the direct algorithm
(HW-measured via repeat-loop marginals, one optimization session spent
trying to beat it):
  - Per-core PE row time: 3072 matmuls x ~500 rows x 1 cyc/row @ 2.4 GHz
    = 638 us.  Instruction count is minimal: free dim (504) is capped by
    the 2 KB PSUM bank, contraction (128) by partitions, out-partitions
    (128) force the m-split.  Measured ~278 ns/matmul = 208 row-time
    + ~62 LDWEIGHTS + ~8 modeled seq overhead.
  - LDWEIGHTS is emitted per-matmul by legalization (LDW+MM pairs,
    ldweights=False on the MM already); cost is ~2 B/cycle/partition and
    CANNOT be elided: repeated-stationary streaks measured SLOWER (+95
    ns/inst), ldweights-field suppression is ignored, and stationary
    reuse across gather tiles (T=2/4 interleave) pays a larger PSUM
    bank-switch penalty (+22..45 ns/inst) than the LDW it saves.
  - fp8e4 DoubleRow (256-contraction/inst) runs at 1 cyc/row on HW (2x
    fp16 MACs/inst; the CoreSim 0.5 cyc/row model is optimistic), but
    plain fp8 fails the 2e-2 gate (~8e-2) and error-corrected fp8
    (wh@xh + wh@xl + wl@xh, all-scale-1024 single PSUM group, rel err
    2.9e-3) needs 1.5x the MACs -> measured 1121-1530 us.  Dead.
  - Interleaved token order (f = 4t+s; gives 3-dim DR moving APs) makes
    the max-reduce stride-4 over PSUM: +53 ns/matmul equivalent.  Dead.
  - The c-outer/j-inner matmul order below is load-bearing: consecutive
    instructions then read overlapping windows of the SAME x chunk.
    j-outer/c-inner (chunk hop every instruction) measured 6.1% slower
    in a drift-immune interleaved A/B.  Keep c outer.
  - Axon HW timing drifts ~60% across hours (same binary: 503-977 us);
    only back-to-back or interleaved comparisons are meaningful.
  - Winograd F(2,3)/F(4,3) would cut PE rows 1.5-2.3x but the input/
    output transforms land ~300-600 us on DVE/Act; marginal at best.

Second optimization session: vocab-precompute table (T = embed @ W_cat,
conv -> 6KB/token dma_gather + DVE tap-sum/max) investigated end-to-end
and DEAD on this platform; measured facts for the record (probe*.py,
kernel2/3.py, _transcript):
  - addr_space="Shared" DRAM is shared only within an HBM PAIR (cores
    2k,2k+1) on trn2 LNC1 (probe-verified by racing writes).  8-way table
    sharing needs ~140MB/pair cross-pair replication -> dead; pair-level
    dedup (32768 inst -> ~19.9K uniques) only cuts PE to ~410us/core and
    core-local dedup (16384 -> ~12.6K) to ~510us -- TimelineSim marginal
    ~710us, no better than this direct kernel.
  - dma_gather(transpose=True, 6KB rows): 183 GB/s on 1 SWDGE queue, 259
    GB/s round-robin on 4 (num_swdge_queues=4); transpose=False reaches
    343; plain DMA 389.  The 100MB/core table gather alone is ~390us.
  - Manual .then_inc/wait_ge semaphores inside TileContext hang the
    device (probe D3/D4); ordering must use add_dep_helper edges or AP
    deps.  Raw-bass (no TC) sems work (probe P1).
  - dma_start(cond=reg) wedges the device on BOTH the SP HWDGE queue
    (probe E1) and the gpsimd/SWDGE path (kernel3 full-mode rerun on a
    healthy device, second session block).  With cond-DMA dead, SPMD
    cores have NO working primitive to write disjoint halves of a
    pair-Shared table; pair-level dedup is unreachable.
  - Pair AllReduce [[0,1],[2,3],[4,5],[6,7]] works inside TC if NO dep
    edges/then_inc touch the collective instruction itself; order via
    AP deps on its in/out DRAM buffers (probe C).
"""

from contextlib import ExitStack

import numpy as np

import concourse.mybir as mybir
import concourse.tile as tile
from concourse import bacc
from concourse.bass_utils import run_bass_kernel_spmd
from concourse.masks import make_identity

# Problem constants (hardcoded per harness contract).
B, S, L, D, F, V = 16, 64, 128, 512, 256, 30000
N_CORES = 8
NSAMP = B * S // N_CORES          # 128 samples per core
KS = (3, 4, 5)                    # conv kernel sizes
SLOT_BASE = (0, 3, 7)             # tap-slot offsets for conv3/4/5 (12 total)
N_SLOTS = 12
SPG = 4                           # samples per gather tile (512 idxs; >512 crashes the SWDGE transpose-gather)
NGT = NSAMP // SPG                # gather tiles per core
GPT = SPG // 4                    # groups of 4 samples per gather tile

f16 = mybir.dt.bfloat16  # bf16: PE streams it 25% faster than fp16 (interleaved A/B 733 vs 974us); rel err 4.5e-3 vs gate 2e-2
f32 = mybir.dt.float32
i16 = mybir.dt.int16


def build_nc(nsamp=NSAMP, spg=SPG, mode="full"):
    """Build the per-core Bass program (SPMD: same program, 8 cores).

    mode: "full" | "nogather" (memset x tiles) | "nomm" (skip matmul/reduce)
    -- the reduced modes exist only to attribute wall-clock time.
    """
    ngt = nsamp // spg
    gpt = spg // 4
    t_tot = nsamp * L                  # tokens per core
    n_idx = spg * L                    # tokens per gather
    idx_cols_per_gather = n_idx // 16

    nc = bacc.Bacc("TRN2", target_bir_lowering=False, debug=False,
                   num_devices=N_CORES)

    emb_h = nc.dram_tensor("emb", [V, D], f16, kind="ExternalInput")
    idx_h = nc.dram_tensor("idx", [128, t_tot // 16], i16, kind="ExternalInput")
    wst_h = nc.dram_tensor("wst", [128, N_SLOTS, 4, 2, 128], f16,
                           kind="ExternalInput")
    bias_h = nc.dram_tensor("bias", [128, 6], f32, kind="ExternalInput")
    out_h = nc.dram_tensor("out", [nsamp, 3 * F], f32, kind="ExternalOutput")

    with tile.TileContext(nc) as tc, ExitStack() as ctx:
        cpool = ctx.enter_context(tc.tile_pool(name="consts", bufs=1))
        xpool = ctx.enter_context(tc.tile_pool(name="x", bufs=6))
        pspool = ctx.enter_context(
            tc.tile_pool(name="ps", bufs=6, space="PSUM"))
        tppool = ctx.enter_context(
            tc.tile_pool(name="tp", bufs=2, space="PSUM"))

        idx_sb = cpool.tile([128, t_tot // 16], i16)
        w_sb = cpool.tile([128, N_SLOTS, 4, 2, 128], f16)
        bias_sb = cpool.tile([128, 6], f32)
        ident = cpool.tile([128, 128], f32)
        out_sb = cpool.tile([128, 6, nsamp], f32)
        out_t = cpool.tile([nsamp, 6 * 128], f32)

        nc.sync.dma_start(out=idx_sb[:], in_=idx_h.ap()[:])
        nc.sync.dma_start(out=w_sb[:], in_=wst_h.ap()[:])
        nc.sync.dma_start(out=bias_sb[:], in_=bias_h.ap()[:])
        make_identity(nc, ident[:])

        if mode == "nomm":
            nc.gpsimd.memset(out_sb[:], 0.0)
        reps = int(mode[len("repeat"):]) if mode.startswith("repeat") else 0
        loop_cm = tc.For_i(0, reps, 1) if reps else None
        if loop_cm is not None:
            loop_cm.__enter__()
        # Process gather tiles in batches of QB so each stationary weight tile
        # is reused across QB matmuls (amortizes LDWEIGHTS 4x).
        # QB>1 (stationary reuse across gather tiles) measured ~28% SLOWER on
        # HW than back-to-back same-bank matmuls: LDWEIGHTS is already hidden
        # by the PE reorder window + dual SBUF read ports, and interleaving
        # PSUM banks/operand buffers per instruction costs more than it saves.
        QB = 1
        for sup in range(ngt // QB):
            xvs = []
            for q in range(QB):
                t = sup * QB + q
                xt = xpool.tile([128, 4, n_idx], f16, tag="xt")
                if mode == "nogather":
                    nc.gpsimd.memset(xt[:], 0.0)
                else:
                    nc.gpsimd.dma_gather(
                        out_ap=xt[:],
                        in_ap=emb_h.ap()[:],
                        idxs_ap=idx_sb[:, t * idx_cols_per_gather:
                                       (t + 1) * idx_cols_per_gather],
                        num_idxs=n_idx,
                        num_idxs_reg=n_idx,
                        elem_size=D,
                        transpose=True,
                    )
                if mode == "nomm":
                    nc.vector.tensor_copy(out_sb[:, 0, t:t + 1], xt[:, 0, :1])
                xvs.append(xt.rearrange("p c (s l) -> p c s l", s=spg))
            if mode == "nomm":
                continue
            for k_idx, K in enumerate(KS):
                lout = L - K + 1
                for m in range(2):
                    pss = [pspool.tile([128, 4, lout], f32, tag="ps",
                                       name=f"ps_{sup}_{k_idx}_{m}_{q}")
                           for q in range(QB)]
                    n_mm = 4 * K
                    mm = 0
                    for c in range(4):
                        for j in range(K):
                            for q in range(QB):
                                nc.tensor.matmul(
                                    pss[q][:],
                                    w_sb[:, SLOT_BASE[k_idx] + j, c, m, :],
                                    xvs[q][:, c, 0:4, j:j + lout],
                                    start=(mm == 0),
                                    stop=(mm == n_mm - 1),
                                )
                            mm += 1
                    tile6 = k_idx * 2 + m
                    for q in range(QB):
                        gidx = sup * QB + q
                        nc.vector.reduce_max(
                            out_sb[:, tile6, gidx * 4:gidx * 4 + 4],
                            pss[q][:],
                            axis=mybir.AxisListType.X,
                        )

        if loop_cm is not None:
            loop_cm.__exit__(None, None, None)
        # bias + relu on [f(partition), sample] layout, then PE-transpose so
        # the final DMA writes contiguous [sample, 768] rows.
        for tile6 in range(6):
            nc.vector.tensor_scalar(
                out_sb[:, tile6, :], out_sb[:, tile6, :],
                bias_sb[:, tile6:tile6 + 1], 0.0,
                op0=mybir.AluOpType.add, op1=mybir.AluOpType.max,
            )
            tp = tppool.tile([nsamp, 128], f32, tag="tp")
            nc.tensor.transpose(tp[:], out_sb[:, tile6, :], ident[:])
            nc.vector.tensor_copy(
                out_t[:, tile6 * 128:(tile6 + 1) * 128], tp[:])
        nc.sync.dma_start(out=out_h.ap()[:], in_=out_t[:])

    nc.compile()
    return nc


def prep_inputs(text, embed, w3, b3, w4, b4, w5, b5, nsamp=NSAMP, spg=SPG,
                n_cores=N_CORES):
    """Host-side marshaling: shard text, wrap gather indices, fp16-quantize
    and retile the weights/embedding."""
    text = np.ascontiguousarray(np.asarray(text).reshape(B * S, L))
    assert text.max() < V and text.min() >= 0
    import ml_dtypes
    emb16 = np.ascontiguousarray(np.asarray(embed, dtype=np.float32).astype(ml_dtypes.bfloat16))

    wst = np.zeros((128, N_SLOTS, 4, 2, 128), np.float32)
    for k_idx, w in enumerate((w3, w4, w5)):
        w = np.asarray(w, dtype=np.float32)
        for j in range(KS[k_idx]):
            # wst[dd, slot, c, m, ff] = w[m*128+ff, c*128+dd, j]
            wj = w[:, :, j].reshape(2, 128, 4, 128)      # [m, ff, c, dd]
            wst[:, SLOT_BASE[k_idx] + j] = wj.transpose(3, 2, 0, 1)
    wst = np.ascontiguousarray(wst.astype(ml_dtypes.bfloat16))

    bias = np.zeros((128, 6), np.float32)
    for k_idx, b in enumerate((b3, b4, b5)):
        bias[:, 2 * k_idx:2 * k_idx + 2] = \
            np.asarray(b, dtype=np.float32).reshape(2, 128).T
    bias = np.ascontiguousarray(bias)

    ngt = nsamp // spg
    in_maps = []
    for r in range(n_cores):
        tcore = text[r * nsamp:(r + 1) * nsamp].astype(np.int16)
        # token i of gather tile t -> partition i%16, column t*(spg*L/16)+i//16;
        # the 16-row block must be replicated to all 128 partitions (each of
        # the 8 gpsimd sub-cores reads its own 16-partition stripe).
        a = tcore.reshape(ngt, spg * L // 16, 16)         # [t, c, p]
        idx = np.tile(a.transpose(2, 0, 1).reshape(16, -1), (8, 1))
        in_maps.append({
            "emb": emb16,
            "idx": np.ascontiguousarray(idx),
            "wst": wst,
            "bias": bias,
        })
    return in_maps


_CACHE = {}


def kernel(text, embed, w3, b3, w4, b4, w5, b5):
    if "nc" not in _CACHE:
        _CACHE["nc"] = build_nc()
    nc = _CACHE["nc"]
    in_maps = prep_inputs(text, embed, w3, b3, w4, b4, w5, b5)
    res = run_bass_kernel_spmd(nc, in_maps, list(range(N_CORES)))
    out = np.concatenate([res.results[r]["out"] for r in range(N_CORES)],
                         axis=0)
    return out.reshape(B, S, 3 * F).astype(np.float32)

